# revision 1
# baseline (speedup 1.0000x reference)
"""Trainium2 Bass kernel for a ViT attention block (LN->MHA+relpos->LN->MLP).

Contract: kernel(**inputs) takes the FULL unsharded inputs, shards batch
across 8 NeuronCores (4 items per core), runs one SPMD Bass program, and
gathers the full [32, 577, 768] fp32 output.

Design notes (v3)
- fp8e4 (e4m3) DoubleRow matmuls (2x PE rate) for qkv, v, P@v, proj and
  fc1; weights quantized host-side at x128 scale (clipped to TRN's +-240),
  descaled at each PSUM readout. S (q.k) and fc2 stay bf16. Measured
  rel_l2 1.33e-2 vs the fp32 reference (gate 2e-2), matching the numpy
  quantization simulation exactly.
- LayerNorm gamma/beta folded into following matmul weights on the host;
  rstd computed as exp(-0.5*ln(var+eps)) and the activation-table list is
  patched during compile so ln/exp co-reside in one table -> no
  ACT_TABLE_LOAD thrash in the attention phase.
- Relative-position bias (log space, fp8) is accumulated into the S PSUM
  by a DoubleRow identity matmul (subtile 1 zeros, 0-stride rhs subtile);
  exp reads PSUM and writes the fp8 P^T tile directly.
- kz slab is head-pair packed; S matmuls contract over 64 partitions at
  base 0/64. v slab is [128, 5, 12, 128] (head pitch 128) because dual-fp8
  LDWEIGHTS rejects non-128-aligned stationary strides.
- Softmax denominators via the ones-column trick (den/16 for fp8 range);
  reciprocal on vector, DRAM-broadcast, normalize into the fp8 O slab on
  GPSIMD (vector for the last item where latency is exposed).
- Tokens padded 577 -> 640; padded keys get -30 bias so exp ~= 0.
- Item 0's LN chain is emitted before all weight DMAs (queue-order
  startup fix); xh2 transposed + fp8-converted per item into persistent
  slabs so the MLP phase starts without a transition bubble; fc1 runs at
  512-wide moving tiles.
"""

import sys

if '/opt/trn_rl_repo' not in sys.path:
    sys.path.insert(0, '/opt/trn_rl_repo')

from contextlib import ExitStack

import numpy as np
import ml_dtypes

import concourse.bass as bass  # noqa: F401
import concourse.tile as tile
import concourse.mybir as mybir
from concourse import bacc, bass_utils

BF16 = ml_dtypes.bfloat16
FP8 = ml_dtypes.float8_e4m3fn
F32 = np.float32

B = 32
N = 577
C = 768
NH = 12
HD = 64
MLP = 3072
EPS = 1e-6
SCALE = HD ** (-0.5)

N_CORES = 8
BPC = B // N_CORES          # 4 batch items per core
NPAD = 640                  # per-item padded token count (5 * 128)
TOK = BPC * NPAD            # 2560 padded tokens per core
KC = C // 128               # 6 contraction chunks for dim 768
MC = MLP // 128             # 24 chunks for MLP dim
MCHUNK = NPAD // 128        # 5 m-chunks per batch item
F32T = mybir.dt.float32
BF16T = mybir.dt.bfloat16
FP8T = mybir.dt.float8e4
DR = mybir.MatmulPerfMode.DoubleRow
AF = mybir.ActivationFunctionType
OP = mybir.AluOpType
WS = 128.0   # fp8 host weight scale
OS = 16.0    # fp8 O (attention output) scale

SPLITS_N = [(0, 512), (512, 65)]   # 577-wide outputs (PSUM bank = 512 fp32)
SPLITS_C = [(0, 512), (512, 256)]  # 768-wide outputs (bank-aligned)

ZERO_ALL_SLABS = False  # sim-only: defeat the pool-slot zero-persistence


def _ln_stats(nc, pool, xt, eps_sb):
    """Per-token mean/rstd for a [128, C] fp32 chunk -> (mean_ap, rstd_ap).

    rstd = exp(-0.5 * ln(var + eps)): ln and exp share one activation table,
    so the attention phase never reloads tables.
    """
    st = pool.tile([128, 2, 6], F32T, tag="bnst")
    nc.vector.bn_stats(st[:, 0, :], xt[:, 0:C // 2])
    nc.vector.bn_stats(st[:, 1, :], xt[:, C // 2:C])
    mv = pool.tile([128, 2], F32T, tag="bnmv")
    nc.vector.bn_aggr(mv[:], st[:])
    sd = pool.tile([128, 1], F32T, tag="sd")
    nc.scalar.activation(sd[:], mv[:, 1:2], AF.Ln, bias=eps_sb[:, 0:1])
    rstd = pool.tile([128, 1], F32T, tag="rstd")
    nc.scalar.activation(rstd[:], sd[:], AF.Exp, scale=-0.5)
    return mv, rstd


def build_program(nc):
    dt = mybir.dt

    x_d = nc.dram_tensor("x", [TOK, C], dt.float32, kind="ExternalInput")
    xb_d = nc.dram_tensor("xb", [TOK, C], dt.float32, kind="ExternalInput")
    wqk_d = nc.dram_tensor("wqkT", [C, 2 * C], dt.float8e4, kind="ExternalInput")
    bqk_d = nc.dram_tensor("bias_qk", [2 * C], dt.float32, kind="ExternalInput")
    wv_d = nc.dram_tensor("wvT", [C, C], dt.float8e4, kind="ExternalInput")
    bv_d = nc.dram_tensor("bias_v", [C], dt.bfloat16, kind="ExternalInput")
    wp_d = nc.dram_tensor("wprojT", [C, C], dt.float8e4, kind="ExternalInput")
    w1_d = nc.dram_tensor("w1T", [C, MLP], dt.float8e4, kind="ExternalInput")
    b1_d = nc.dram_tensor("bias_fc1", [MLP], dt.float32, kind="ExternalInput")
    w2_d = nc.dram_tensor("w2T", [MLP, C], dt.bfloat16, kind="ExternalInput")
    b2_d = nc.dram_tensor("bias_fc2", [C], dt.bfloat16, kind="ExternalInput")
    rpb8_d = nc.dram_tensor("rpb8", [NH, NPAD, N], dt.float8e4,
                            kind="ExternalInput")
    ident8_d = nc.dram_tensor("ident8", [128, 2, 128], dt.float8e4,
                              kind="ExternalInput")
    out_d = nc.dram_tensor("out", [TOK, C], dt.float32, kind="ExternalOutput")

    xh_d = nc.dram_tensor("xh_scratch", [TOK, C], dt.bfloat16)
    xh2_d = nc.dram_tensor("xh2_scratch", [TOK, C], dt.bfloat16)
    x2_d = nc.dram_tensor("x2_scratch", [TOK, C], dt.float32)
    rec_d = nc.dram_tensor("rec_scratch", [BPC, NH, N], dt.float32)

    x_ap = x_d.ap().rearrange("(c p) d -> p c d", p=128)      # [128, 20, 768]
    xb_ap = xb_d.ap().rearrange("(c p) d -> p c d", p=128)
    xh_ap = xh_d.ap().rearrange("(c p) d -> p c d", p=128)
    xh2_ap = xh2_d.ap().rearrange("(c p) d -> p c d", p=128)
    x2_ap = x2_d.ap().rearrange("(c p) d -> p c d", p=128)
    out_ap = out_d.ap().rearrange("(c p) d -> p c d", p=128)

    with tile.TileContext(nc) as tc, ExitStack() as ctx:
        persist = ctx.enter_context(tc.tile_pool(name="persist", bufs=1))
        psum = ctx.enter_context(tc.tile_pool(name="psum", bufs=4, space="PSUM"))

        eps_sb = persist.tile([128, 1], F32T, tag="eps")
        nc.vector.memset(eps_sb[:], EPS)
        bqk_sb = persist.tile([128, 12], F32T, tag="bqk")
        bv_sb = persist.tile([128, C], BF16T, tag="bv")
        bfc1_sb = persist.tile([128, MC], F32T, tag="bfc1")
        bfc2_sb = persist.tile([128, C], BF16T, tag="bfc2")

        def emit_persist_dmas():
            # issued after item 0's LN chain so its x DMAs + transposes are
            # first in every DMA queue (kills the startup bubble)
            nc.sync.dma_start(bqk_sb[:],
                              bqk_d.ap().rearrange("(m p) -> p m", p=128))
            bvsrc = bv_d.ap()
            nc.sync.dma_start(bv_sb[:], bass.AP(
                tensor=bvsrc.tensor, offset=bvsrc.offset,
                ap=[[0, 128]] + list(bvsrc.ap)))
            nc.sync.dma_start(bfc1_sb[:],
                              b1_d.ap().rearrange("(m p) -> p m", p=128))
            b2src = b2_d.ap()
            nc.sync.dma_start(bfc2_sb[:], bass.AP(
                tensor=b2src.tensor, offset=b2src.offset,
                ap=[[0, 128]] + list(b2src.ap)))
            nc.sync.dma_start(ident8[:], ident8_d.ap())
        ident8 = persist.tile([128, 2, 128], FP8T, tag="ident8")
        # persistent slab receiving per-item LN2 transposes for the MLP
        xh2T_all = persist.tile([128, KC, TOK], BF16T, tag="xh2T")
        xh2T8_all = persist.tile([128, KC, TOK], FP8T, tag="xh2T8")

        # ---------- attention-superphase scope ----------
        abc_ctx = ExitStack()
        ap_w = abc_ctx.enter_context(tc.tile_pool(name="attnw", bufs=1))
        wp_sb = ap_w.tile([128, KC, C], FP8T, tag="wp")
        wqk_sb = ap_w.tile([128, KC, 2 * C], FP8T, tag="wqk")
        wv_sb = ap_w.tile([128, KC, C], FP8T, tag="wv")

        def emit_weight_dmas():
            nc.sync.dma_start(
                wqk_sb[:], wqk_d.ap().rearrange("(k p) c -> p k c", p=128))
            nc.sync.dma_start(
                wv_sb[:], wv_d.ap().rearrange("(k p) c -> p k c", p=128))
            nc.sync.dma_start(
                wp_sb[:], wp_d.ap().rearrange("(k p) c -> p k c", p=128))

        # per-b double-buffered big slabs
        qkp = abc_ctx.enter_context(tc.tile_pool(name="qkp", bufs=2))
        kzp = abc_ctx.enter_context(tc.tile_pool(name="kzp", bufs=2))
        vp = abc_ctx.enter_context(tc.tile_pool(name="vp", bufs=2))
        xhp = abc_ctx.enter_context(tc.tile_pool(name="xhp", bufs=2))
        xhp8 = abc_ctx.enter_context(tc.tile_pool(name="xhp8", bufs=2))
        o8p = abc_ctx.enter_context(tc.tile_pool(name="o8p", bufs=2))
        rpbp = abc_ctx.enter_context(tc.tile_pool(name="rpb", bufs=2))
        ptp = abc_ctx.enter_context(tc.tile_pool(name="pt", bufs=2))
        smallp = abc_ctx.enter_context(tc.tile_pool(name="attnsmall", bufs=3))
        rbp = abc_ctx.enter_context(tc.tile_pool(name="rbp", bufs=2))
        ck = abc_ctx.enter_context(tc.tile_pool(name="lnck", bufs=3))
        ck2 = abc_ctx.enter_context(tc.tile_pool(name="projck", bufs=2))

        def emit_ln1_b(b):
            """LN1 for item b's 5 token chunks -> xh_dram, then transpose
            into a fresh per-b xhT slab (two channel-half transposes so the
            first qkv matmuls can start after the first half lands)."""
            for i in range(b * MCHUNK, (b + 1) * MCHUNK):
                xt = ck.tile([128, C], F32T, tag="xt")
                nc.sync.dma_start(xt[:], x_ap[:, i, :])
                mv, rstd = _ln_stats(nc, ck, xt, eps_sb)
                xh_t = ck.tile([128, C], BF16T, tag="xh")
                nc.vector.tensor_scalar(
                    out=xh_t[:], in0=xt[:], scalar1=mv[:, 0:1],
                    scalar2=rstd[:, 0:1], op0=OP.subtract, op1=OP.mult)
                nc.sync.dma_start(xh_ap[:, i, :], xh_t[:])
            xhT = xhp.tile([128, KC, NPAD], BF16T, tag="xhT")
            nc.sync.dma_start_transpose(
                xhT[:, 0:KC // 2, :],
                xh_d.ap()[b * NPAD:(b + 1) * NPAD, 0:C // 2])
            nc.sync.dma_start_transpose(
                xhT[:, KC // 2:KC, :],
                xh_d.ap()[b * NPAD:(b + 1) * NPAD, C // 2:C])
            xhT8 = xhp8.tile([128, KC, NPAD], FP8T, tag="xhT8")
            nc.vector.tensor_scalar(
                out=xhT8[:, 0:KC // 2, :], in0=xhT[:, 0:KC // 2, :],
                scalar1=1.0, scalar2=None, op0=OP.mult)
            nc.vector.tensor_scalar(
                out=xhT8[:, KC // 2:KC, :], in0=xhT[:, KC // 2:KC, :],
                scalar1=1.0, scalar2=None, op0=OP.mult)
            return xhT8

        def emit_s_chunk(qkT, kz, hp, e, pt, mc, e_tile):
            """One S^T chunk for head h=2hp+e: 64-wide bf16 q.k contraction,
            then the fp8 log-space rel-pos bias accumulated into the same
            PSUM via a DoubleRow identity matmul (subtile 1 of ident8 is
            zeros, the 0-stride rhs subtile is ignored), then exp straight
            from PSUM into the fp8 P^T tile."""
            sps = psum.tile([128, 768], F32T, tag="ps")
            for (lo, w) in SPLITS_N:
                esl = e_tile[:, mc, lo:lo + w]
                erhs = bass.AP(tensor=esl.tensor, offset=esl.offset,
                               ap=[list(esl.ap)[0], [0, 2], list(esl.ap)[-1]])
                nc.tensor.matmul(
                    sps[:, lo:lo + w],
                    lhsT=ident8[:], rhs=erhs,
                    start=True, stop=False, perf_mode=DR)
                nc.tensor.matmul(
                    sps[:, lo:lo + w],
                    lhsT=kz[64 * e:64 * e + 64, hp,
                            mc * 128:(mc + 1) * 128],
                    rhs=qkT[64 * e:64 * e + 64, hp, lo:lo + w],
                    start=False, stop=True)
            nc.scalar.activation(pt[:, mc, :], sps[:, 0:N], AF.Exp)

        class PvCtx:
            """Pending P^T @ [v | 1] for one head, drained a few matmuls at a
            time between the next head's S chunks. On completion the
            denominator row goes to den12[h] and unnormalized O^T goes
            straight into the kz slab slot head h vacated."""

            def __init__(self, pt, v_sb, h, den12, kz):
                self.pt, self.v_sb, self.h = pt, v_sb, h
                self.den12, self.kz = den12, kz
                self.pv = psum.tile([128, 768], F32T, tag="ps")
                self.mms = [(lo, w, j) for (lo, w) in SPLITS_N
                            for j in range(3)]
                self.pos = 0

            def drain(self, k):
                end = min(self.pos + k, len(self.mms))
                for (lo, w, j) in self.mms[self.pos:end]:
                    if j < 2:
                        nc.tensor.matmul(
                            self.pv[:, lo:lo + w],
                            lhsT=self.v_sb[0:128, 2 * j:2 * j + 2,
                                           self.h, 0:128],
                            rhs=self.pt[0:128, 2 * j:2 * j + 2, lo:lo + w],
                            start=(j == 0), stop=False, perf_mode=DR)
                    else:
                        nc.tensor.matmul(
                            self.pv[:, lo:lo + w],
                            lhsT=self.v_sb[0:128, 4, self.h, 0:128],
                            rhs=self.pt[0:128, 4, lo:lo + w],
                            start=False, stop=True)
                self.pos = end
                if self.pos == len(self.mms):
                    h = self.h
                    dd = smallp.tile([1, N], F32T, tag="dd")
                    nc.vector.tensor_scalar(
                        out=dd[:], in0=self.pv[64:65, 0:N],
                        scalar1=1.0 / OS, scalar2=None, op0=OP.mult)
                    nc.sync.dma_start(self.den12[h:h + 1, :], dd[:])
                    nc.vector.tensor_copy(
                        self.kz[64 * (h % 2):64 * (h % 2) + 64, h // 2, 0:N],
                        self.pv[0:64, 0:N])
                    self.pv = None
                    return True
                return False

            def finish(self):
                while self.pv is not None:
                    self.drain(4)

        def emit_proj_chunk(o8, b, ic, half):
            """One half (512 or 256 cols) of proj+residual+LN2 for chunk ic.
            half=0 emits the 512 split; half=1 emits the 256 split plus the
            residual/LN2 tail."""
            i = b * MCHUNK + ic
            if half == 0:
                ps = psum.tile([128, 768], F32T, tag="ps")
                proj_ps[ic] = ps
            else:
                ps = proj_ps.pop(ic)
            (lo, w) = SPLITS_C[half]
            for j in range(KC // 2):
                nc.tensor.matmul(
                    ps[:, lo:lo + w],
                    lhsT=o8[:, 2 * j:2 * j + 2, ic * 128:(ic + 1) * 128],
                    rhs=wp_sb[:, 2 * j:2 * j + 2, lo:lo + w],
                    start=(j == 0), stop=(j == KC // 2 - 1), perf_mode=DR)
            if half == 0:
                return
            xt = ck2.tile([128, C], F32T, tag="xt2")
            nc.sync.dma_start(xt[:], xb_ap[:, i, :])
            x2t = ck2.tile([128, C], F32T, tag="x2t")
            nc.vector.scalar_tensor_tensor(
                out=x2t[:], in0=ps[:, 0:C], scalar=1.0 / (WS * OS),
                in1=xt[:], op0=OP.mult, op1=OP.add)
            nc.sync.dma_start(x2_ap[:, i, :], x2t[:])
            mv, rstd = _ln_stats(nc, ck2, x2t, eps_sb)
            xh2t = ck2.tile([128, C], BF16T, tag="xh2")
            nc.vector.tensor_scalar(
                out=xh2t[:], in0=x2t[:], scalar1=mv[:, 0:1],
                scalar2=rstd[:, 0:1], op0=OP.subtract, op1=OP.mult)
            nc.sync.dma_start(xh2_ap[:, i, :], xh2t[:])

        proj_ps = {}

        def finalize_steps(b, kz, o8, den12):
            """Secondary-step closures for item b: reciprocal, 12 GPSIMD
            broadcast+normalize steps, 10 proj half-chunks, then the item's
            xh2T transpose into the persistent MLP slab."""
            steps = []
            cell = {}

            def recip_step():
                rec12 = smallp.tile([12, N], F32T, tag="rec")
                nc.vector.reciprocal(rec12[:], den12[:])
                nc.sync.dma_start(rec_d.ap()[b], rec12[:])
            steps.append(recip_step)

            eng = nc.vector if b == BPC - 1 else nc.gpsimd

            def mult_step(h):
                # [128, N] broadcast so the in1 base partition matches the
                # kz slice for either head parity (walrus samePartitionsAll).
                rb = rbp.tile([128, N], F32T, tag="rb")
                rsrc = rec_d.ap()[b, h]
                nc.sync.dma_start(rb[:], bass.AP(
                    tensor=rsrc.tensor, offset=rsrc.offset,
                    ap=[[0, 128]] + list(rsrc.ap)))
                base = 64 * (h % 2)
                eng.tensor_tensor(
                    o8[base:base + 64, h // 2, 0:N],
                    kz[base:base + 64, h // 2, 0:N],
                    rb[base:base + 64, :], OP.mult)
            for h in range(NH):
                steps.append(lambda h=h: mult_step(h))
            for ic in range(MCHUNK):
                for half in range(2):
                    steps.append(
                        lambda ic=ic, half=half: emit_proj_chunk(
                            o8, b, ic, half))

            def xh2t_step():
                nc.sync.dma_start_transpose(
                    xh2T_all[:, 0:KC // 2, b * NPAD:(b + 1) * NPAD],
                    xh2_d.ap()[b * NPAD:(b + 1) * NPAD, 0:C // 2])
                nc.sync.dma_start_transpose(
                    xh2T_all[:, KC // 2:KC, b * NPAD:(b + 1) * NPAD],
                    xh2_d.ap()[b * NPAD:(b + 1) * NPAD, C // 2:C])
            steps.append(xh2t_step)

            def xh2t8_step():
                sl = slice(b * NPAD, (b + 1) * NPAD)
                nc.vector.tensor_scalar(
                    out=xh2T8_all[:, 0:KC // 2, sl],
                    in0=xh2T_all[:, 0:KC // 2, sl],
                    scalar1=1.0, scalar2=None, op0=OP.mult)
                nc.vector.tensor_scalar(
                    out=xh2T8_all[:, KC // 2:KC, sl],
                    in0=xh2T_all[:, KC // 2:KC, sl],
                    scalar1=1.0, scalar2=None, op0=OP.mult)
            steps.append(xh2t8_step)
            return steps

        def qkv_steps(b, xhT):  # noqa: b used in alloc_step
            """Secondary-step closures computing q/k/v for item b into fresh
            per-b slabs. Returns (steps, result_cell)."""
            cell = {}

            def alloc_step():
                qkT = qkp.tile([128, KC, N], BF16T, tag="qkT")
                kz = kzp.tile([128, KC, NPAD], BF16T, tag="kz")
                cell['kz'] = kz
                o8 = o8p.tile([128, KC, NPAD], FP8T, tag="o8")
                cell['o8'] = o8
                v_sb = vp.tile([128, MCHUNK, NH, 128], FP8T, tag="v")
                if b < 2 or ZERO_ALL_SLABS:
                    # pool slots alternate; the constant regions are never
                    # overwritten by data, so initializing the first two
                    # slabs covers all four items: kz pad lanes stay 0,
                    # ones columns for the denominator trick stay 1.
                    nc.vector.memset(o8[:], 0.0)
                    nc.vector.memset(kz[:], 0.0)
                    nc.vector.memset(v_sb[:], 0.0)
                    nc.vector.memset(v_sb[:, :, :, 64:66], 1.0)
                cell['qkT'], cell['v'] = qkT, v_sb

            qk_ps = {}

            def qk_step(oc, half):
                if half == 0:
                    ps = psum.tile([128, 768], F32T, tag="ps")
                    qk_ps[oc] = ps
                else:
                    ps = qk_ps.pop(oc)
                (lo, w) = SPLITS_N[half]
                for j in range(KC // 2):
                    nc.tensor.matmul(
                        ps[:, lo:lo + w],
                        lhsT=wqk_sb[:, 2 * j:2 * j + 2,
                                    oc * 128:(oc + 1) * 128],
                        rhs=xhT[:, 2 * j:2 * j + 2, lo:lo + w],
                        start=(j == 0), stop=(j == KC // 2 - 1),
                        perf_mode=DR)
                if half == 1:
                    dst = (cell['qkT'][:, oc, 0:N] if oc < 6
                           else cell['kz'][:, oc - 6, 0:N])
                    nc.vector.tensor_scalar(
                        out=dst, in0=ps[:, 0:N],
                        scalar1=1.0 / WS, scalar2=bqk_sb[:, oc:oc + 1],
                        op0=OP.mult, op1=OP.add)

            v_ps = {}

            def v_step(mc, half):
                mw = 128 if mc < MCHUNK - 1 else N - 4 * 128
                if half == 0:
                    ps = psum.tile([128, 768], F32T, tag="ps")
                    v_ps[mc] = ps
                else:
                    ps = v_ps.pop(mc)
                (lo, w) = SPLITS_C[half]
                for j in range(KC // 2):
                    nc.tensor.matmul(
                        ps[:, lo:lo + w],
                        lhsT=xhT[:, 2 * j:2 * j + 2,
                                 mc * 128:(mc + 1) * 128],
                        rhs=wv_sb[:, 2 * j:2 * j + 2, lo:lo + w],
                        start=(j == 0), stop=(j == KC // 2 - 1),
                        perf_mode=DR)
                if half == 1:
                    nc.vector.scalar_tensor_tensor(
                        out=cell['v'][0:mw, mc, :, 0:64],
                        in0=ps[0:mw, 0:768].rearrange("p (h e) -> p h e", h=NH),
                        scalar=1.0 / WS,
                        in1=bv_sb[0:mw, :].rearrange("p (h e) -> p h e", h=NH),
                        op0=OP.mult, op1=OP.add)

            steps = [alloc_step]
            for oc in range(12):
                steps.append(lambda oc=oc: qk_step(oc, 0))
                steps.append(lambda oc=oc: qk_step(oc, 1))
            for mc in range(MCHUNK):
                steps.append(lambda mc=mc: v_step(mc, 0))
                steps.append(lambda mc=mc: v_step(mc, 1))
            return steps, cell

        # ---------------- main pipelined loop ----------------
        # Prologue: item 0's LN chain first so its DMAs head every queue,
        # then weights, then item 0 qkv wholesale.
        xhT_cur = emit_ln1_b(0)
        emit_weight_dmas()
        emit_persist_dmas()
        q_steps, q_cell = qkv_steps(0, xhT_cur)
        for s in q_steps:
            s()
        cur = (q_cell['qkT'], q_cell['kz'], q_cell['v'], q_cell['o8'])
        states = {}
        pending = [None]
        for b in range(BPC):
            qkT, kz, v_sb, o8 = cur
            den12 = smallp.tile([12, N], F32T, tag="den")
            states[b] = den12
            if b + 1 < BPC:
                xhT_next = emit_ln1_b(b + 1)
            sec = []
            if b - 1 in states:
                sec += finalize_steps(b - 1, prev_kz, prev_o8,
                                      states.pop(b - 1))
            if b + 1 < BPC:
                q_steps, q_cell = qkv_steps(b + 1, xhT_next)
                sec += q_steps
            sec_i = 0
            for head_idx, (hp, e) in enumerate(
                    (hp, e) for hp in range(6) for e in range(2)):
                h = 2 * hp + e
                if e == 0:
                    e_tiles = []
                    for hh in (h, h + 1):
                        rt = rpbp.tile([128, MCHUNK, N], FP8T, tag="rpb")
                        nc.sync.dma_start(
                            rt[:],
                            rpb8_d.ap()[hh].rearrange("(m p) n -> p m n", p=128))
                        e_tiles.append(rt)
                pt = ptp.tile([128, MCHUNK, N], FP8T, tag="pt")
                for mc in range(MCHUNK):
                    emit_s_chunk(qkT, kz, hp, e, pt, mc, e_tiles[e])
                    if pending[0] is not None:
                        if pending[0].drain(2 if mc < MCHUNK - 1 else 4):
                            pending[0] = None
                    if head_idx >= 1 and sec_i < len(sec):
                        sec[sec_i]()
                        sec_i += 1
                if pending[0] is not None:
                    pending[0].finish()
                    pending[0] = None
                pending[0] = PvCtx(pt, v_sb, h, den12, kz)
            # drain remaining secondary steps for this b
            while sec_i < len(sec):
                sec[sec_i]()
                sec_i += 1
            prev_kz, prev_o8 = kz, o8
            if b + 1 < BPC:
                cur = (q_cell['qkT'], q_cell['kz'], q_cell['v'],
                       q_cell['o8'])
        if pending[0] is not None:
            pending[0].finish()
            pending[0] = None
        for s in finalize_steps(BPC - 1, prev_kz, prev_o8,
                                states.pop(BPC - 1)):
            s()

        abc_ctx.close()

        # ================= MLP =================
        with ExitStack() as mctx:
            mlpp = mctx.enter_context(tc.tile_pool(name="mlp", bufs=1))
            w1_sb = mlpp.tile([128, KC, MLP], FP8T, tag="w1")
            w1_src = w1_d.ap().rearrange("(k p) c -> p k c", p=128)
            for kc in range(KC):
                nc.sync.dma_start(w1_sb[:, kc, :], w1_src[:, kc, :])
            w2_sb = mlpp.tile([128, MC, C], BF16T, tag="w2")
            w2_src = w2_d.ap().rearrange("(k p) c -> p k c", p=128)
            for mc8 in range(8):
                nc.sync.dma_start(w2_sb[:, mc8 * 3:(mc8 + 1) * 3, :],
                                  w2_src[:, mc8 * 3:(mc8 + 1) * 3, :])

            mtp = mctx.enter_context(tc.tile_pool(name="mt", bufs=2))
            ck3 = mctx.enter_context(tc.tile_pool(name="mlpck", bufs=3))
            NB = 512
            for nb in range(TOK // NB):
                mt = mtp.tile([128, MC, NB], BF16T, tag="mt")
                for mc in range(MC):
                    mps = psum.tile([128, 768], F32T, tag="ps")
                    for j in range(KC // 2):
                        nc.tensor.matmul(
                            mps[:, 0:NB],
                            lhsT=w1_sb[:, 2 * j:2 * j + 2,
                                       mc * 128:(mc + 1) * 128],
                            rhs=xh2T8_all[:, 2 * j:2 * j + 2,
                                          nb * NB:(nb + 1) * NB],
                            start=(j == 0), stop=(j == KC // 2 - 1),
                            perf_mode=DR)
                    nc.scalar.activation(mt[:, mc, :], mps[:, 0:NB], AF.Gelu,
                                         bias=bfc1_sb[:, mc:mc + 1],
                                         scale=1.0 / WS)
                for ns in range(NB // 128):
                    i = nb * (NB // 128) + ns
                    fps = psum.tile([128, 768], F32T, tag="ps")
                    for (lo, w) in SPLITS_C:
                        for mc in range(MC):
                            nc.tensor.matmul(
                                fps[:, lo:lo + w],
                                lhsT=mt[:, mc, ns * 128:(ns + 1) * 128],
                                rhs=w2_sb[:, mc, lo:lo + w],
                                start=(mc == 0), stop=(mc == MC - 1))
                    xf = ck3.tile([128, C], F32T, tag="xf")
                    nc.sync.dma_start(xf[:], x2_ap[:, i, :])
                    ot = ck3.tile([128, C], F32T, tag="ot")
                    nc.vector.tensor_tensor(ot[:], fps[:, 0:C], xf[:], OP.add)
                    nc.vector.tensor_tensor(ot[:], ot[:], bfc2_sb[:], OP.add)
                    nc.sync.dma_start(out_ap[:, i, :], ot[:])


def host_prep(inputs):
    """Fold layernorms/biases/scale into weights; build per-core input maps."""
    x = np.asarray(inputs['x'], F32)
    qkv_w = np.asarray(inputs['qkv_w'], F32)
    g1 = np.asarray(inputs['norm1_g'], F32)
    b1 = np.asarray(inputs['norm1_b'], F32)
    q_bias = np.asarray(inputs['q_bias'], F32)
    v_bias = np.asarray(inputs['v_bias'], F32)
    rpb_table = np.asarray(inputs['rpb_table'], F32)
    rel_index = np.asarray(inputs['rel_index'])
    proj_w = np.asarray(inputs['proj_w'], F32)
    proj_b = np.asarray(inputs['proj_b'], F32)
    g2 = np.asarray(inputs['norm2_g'], F32)
    b2 = np.asarray(inputs['norm2_b'], F32)
    fc1_w = np.asarray(inputs['fc1_w'], F32)
    fc1_b = np.asarray(inputs['fc1_b'], F32)
    fc2_w = np.asarray(inputs['fc2_w'], F32)
    fc2_b = np.asarray(inputs['fc2_b'], F32)

    Wq = qkv_w[0:C] * g1[None, :] * SCALE
    bias_q = (qkv_w[0:C] @ b1 + q_bias) * SCALE
    Wk = qkv_w[C:2 * C] * g1[None, :]
    bias_k = qkv_w[C:2 * C] @ b1
    Wv = qkv_w[2 * C:] * g1[None, :]
    bias_v = qkv_w[2 * C:] @ b1 + v_bias

    def q8w(a):
        ab = np.ascontiguousarray(a).astype(BF16).astype(F32)
        return np.clip(ab * WS, -240, 240).astype(FP8)

    wqkT = q8w(np.concatenate([Wq, Wk], 0).T)
    bias_qk = np.concatenate([bias_q, bias_k]).astype(F32)
    wvT = q8w(Wv.T)
    wprojT = q8w(proj_w.T)
    w1T = q8w((fc1_w * g2[None, :]).T)
    bias_fc1 = (fc1_w @ b2 + fc1_b).astype(F32)
    w2T = np.ascontiguousarray(fc2_w.T).astype(BF16)

    rpb = rpb_table[rel_index]                     # [N, N, NH]
    rpbT = np.full((NH, NPAD, N), -30.0, F32)      # pad keys -> exp ~= 0
    rpbT[:, :N, :] = rpb.transpose(2, 1, 0)        # rpbT[h, m, n] = rpb[n, m, h]
    rpb8 = np.clip(rpbT, -240, 240).astype(FP8)
    ident8 = np.zeros((128, 2, 128), F32)
    ident8[:, 0, :] = np.eye(128)
    ident8 = ident8.astype(FP8)

    shared = dict(
        wqkT=wqkT, bias_qk=bias_qk, wvT=wvT, bias_v=bias_v.astype(BF16),
        wprojT=wprojT,
        w1T=w1T, bias_fc1=bias_fc1, w2T=w2T, bias_fc2=fc2_b.astype(BF16),
        rpb8=rpb8, ident8=ident8)

    xpad = np.zeros((B, NPAD, C), F32)
    xpad[:, :N, :] = x
    xbpad = xpad + proj_b[None, None, :].astype(F32)
    in_maps = []
    for core in range(N_CORES):
        xi = xpad[core * BPC:(core + 1) * BPC].reshape(TOK, C)
        xbi = xbpad[core * BPC:(core + 1) * BPC].reshape(TOK, C)
        m = dict(shared)
        m['x'] = np.ascontiguousarray(xi)
        m['xb'] = np.ascontiguousarray(xbi)
        in_maps.append(m)
    return in_maps


def _patch_act_tables():
    """Force Exp/Ln to co-reside in the natural_log_exp_and_others table so
    the attention phase never reloads activation tables (Gelu keeps its own
    table for the MLP phase). Table indices are preserved; only membership
    of Exp/Ln in other tables is hidden from the placement pass."""
    import concourse.hw_specs as hw_specs
    import concourse.bacc as bacc_mod
    orig = hw_specs.get_activation_tables

    def patched(arch):
        t = orig(arch)
        for name, funcs in t.items():
            if name != 'natural_log_exp_and_others':
                funcs.discard(AF.Exp)
                funcs.discard(AF.Ln)
        return t

    hw_specs.get_activation_tables = patched
    if getattr(bacc_mod, 'get_activation_tables', None) is orig:
        bacc_mod.get_activation_tables = patched
    return orig


def _unpatch_act_tables(orig):
    import concourse.hw_specs as hw_specs
    import concourse.bacc as bacc_mod
    hw_specs.get_activation_tables = orig
    if getattr(bacc_mod, 'get_activation_tables', None) is not orig:
        bacc_mod.get_activation_tables = orig


def build_bass():
    nc = bacc.Bacc("TRN2", target_bir_lowering=False, debug=False,
                   num_devices=N_CORES)
    build_program(nc)
    orig = _patch_act_tables()
    try:
        nc.compile()
    finally:
        _unpatch_act_tables(orig)
    return nc


def gather_output(results):
    out = np.zeros((B, N, C), F32)
    for core in range(N_CORES):
        o = results[core]["out"].reshape(BPC, NPAD, C)
        out[core * BPC:(core + 1) * BPC] = o[:, :N, :]
    return out


def kernel(**inputs):
    in_maps = host_prep(inputs)
    nc = build_bass()
    res = bass_utils.run_bass_kernel_spmd(nc, in_maps,
                                          core_ids=list(range(N_CORES)))
    return gather_output(res.results)

